# revision 20
# baseline (speedup 1.0000x reference)
"""Trainium2 Bass kernel: multi-head attention with sparsemax (sparse attention).

Problem: nn_MultiHeadAttention_24309514895753
  bs=8, L=1024, d=512, H=8 heads, head dim D=64, fp32.
  out = sparsemax((h_q Wq^T / sqrt(D)) (h_k Wk^T)^T) (h_v Wv^T + bv) Wf^T + bf

Sharding: data-parallel over batch (8 cores, core b owns batch element b).
No collectives needed.

Per-core algorithm (same math as the baseline, restructured for overlap):
  1. Projections on PE in transposed layout: QT[o,l] (pre-scaled by 1/temp),
     KT[o,l], V[l,o].  Bias bv folded into the final bias on the host.
     Input DMAs are chunked; head-0 S matmuls interleave with the
     projection matmuls.
  2. Per head h: S = Q_h K_h^T into PSUM per (q-tile, k-half); DVE max8 ->
     top-8 per 512-half, top-8 (M1) / negated top-8 (M2) of the 16
     candidates.  tau = max_j (cumsum_j - 1)/j over the sorted top-16:
     cumsum/suffix sums on GPSIMD via zero-padded shift-adds, candidate
     formation + reduce on DVE.  tau -> PE transpose -> negate -> row 64
     of the 65-row QT tile (KT row 64 = ones).
  3. S^T - tau (K=65) -> Relu (ACT) -> alpha^T; AV accumulates res^T.
  4. Final projection out^T = Wf res + bias, DMA out^T; host transposes.

All matmuls (S, S^T, AV, projections, tau transpose) are shaped to run in
the PE's full 128x128 tiling mode -- S contracts over K=65 (row 64 of QT
is zero during the S phase, so it adds exact 0), AV uses a 128-column
stationary operand spanning two heads' V (the lower 64 output rows are
ignored), and the tau transpose input is padded to 72 columns.  Mixed
tile modes force PE array drains on every mode switch (~2x matmul cost).

Pipelining (2-deep): head h+2's S matmuls interleave with head h's
S^T/AV matmuls in PE issue order, so head h+1's tau chain (computed from
S(h+1) during slot h-1) is never on the PE critical path.  AV lags S^T by
one k-chunk so Relu latency is hidden.  The tau row DMA is issued from
the scalar engine (the sync queue is busy issuing input-DMA doorbells
early on), and input/output DMAs are chunked/split so transfers spread
across DMA queues and overlap compute.

Matmul dtype: float32r (11-bit mantissa, 4x fp32 rate).  Inputs pre-rounded
to the fp32r grid on the host so S and S^T are bit-consistent.
"""

import numpy as np

N_HEADS = 8
N_DIM = 512
ATTN_DIM = 64
TEMPERATURE = ATTN_DIM ** 0.5
BS = 8
L = 1024

MM_DTYPE_F32R = True

_COMPILED = {}


def _build_nc():
    import concourse.bacc as bacc
    import concourse.mybir as mybir
    import concourse.tile as tile
    from concourse.masks import make_identity

    F32 = mybir.dt.float32
    MMD = mybir.dt.float32r if MM_DTYPE_F32R else F32
    AT = mybir.AluOpType
    AF = mybir.ActivationFunctionType
    AX = mybir.AxisListType

    nc = bacc.Bacc("TRN2", target_bir_lowering=False, debug=False, num_devices=8)

    hqT_d = nc.dram_tensor("hqT", [N_DIM, L], MMD, kind="ExternalInput").ap()
    hkT_d = nc.dram_tensor("hkT", [N_DIM, L], MMD, kind="ExternalInput").ap()
    hvT_d = nc.dram_tensor("hvT", [N_DIM, L], MMD, kind="ExternalInput").ap()
    wqT_d = nc.dram_tensor("wqT", [N_DIM, N_DIM], MMD, kind="ExternalInput").ap()
    wkT_d = nc.dram_tensor("wkT", [N_DIM, N_DIM], MMD, kind="ExternalInput").ap()
    wvT_d = nc.dram_tensor("wvT", [N_DIM, N_DIM], MMD, kind="ExternalInput").ap()
    wfT_d = nc.dram_tensor("wfT", [N_DIM, N_DIM], MMD, kind="ExternalInput").ap()
    bf2_d = nc.dram_tensor("bf2", [N_DIM], F32, kind="ExternalInput").ap()
    rec_d = nc.dram_tensor("recj", [128, 32], F32, kind="ExternalInput").ap()
    outT_d = nc.dram_tensor("outT", [N_DIM, L], F32, kind="ExternalOutput").ap()

    H = N_HEADS
    NQT = L // 128          # 8 q tiles per head
    NKC = L // 128          # 8 k chunks per head
    NDC = N_DIM // 128      # 4 feature chunks
    VW = N_DIM + 64         # v_s free width: 64 zero-pad cols for head 7's AV

    with tile.TileContext(nc) as tc:
        with tc.tile_pool(name="pW", bufs=1) as pW, \
             tc.tile_pool(name="pQK", bufs=1) as pQK, \
             tc.tile_pool(name="pV", bufs=1) as pV, \
             tc.tile_pool(name="pRes", bufs=1) as pRes, \
             tc.tile_pool(name="pOut", bufs=4) as pOut, \
             tc.tile_pool(name="pSm", bufs=1) as pSm, \
             tc.tile_pool(name="pWk", bufs=2) as pWk, \
             tc.tile_pool(name="pA", bufs=3) as pA, \
             tc.tile_pool(name="pIn", bufs=1) as pIn, \
             tc.tile_pool(name="pw3", bufs=1) as pw3, \
             tc.tile_pool(name="psA", bufs=2, space="PSUM") as psA, \
             tc.tile_pool(name="psB", bufs=3, space="PSUM") as psB, \
             tc.tile_pool(name="psR", bufs=1, space="PSUM") as psR, \
             tc.tile_pool(name="psT", bufs=1, space="PSUM") as psT:

            # ---- small constants first ----
            recj = pW.tile([128, 32], F32)
            nc.sync.dma_start(out=recj, in_=rec_d)
            bf2_s = pW.tile([128, NDC], F32)
            nc.sync.dma_start(out=bf2_s, in_=bf2_d.rearrange("(m p) -> p m", p=128))
            identity = pW.tile([128, 128], F32)
            make_identity(nc, identity)

            # ---- input staging (all DMAs issued up front, chunked) ----
            hq_s = pIn.tile([128, NDC, L], MMD)
            hk_s = pIn.tile([128, NDC, L], MMD)
            hv_s = pIn.tile([128, NDC, L], MMD)
            wq_s = pw3.tile([128, NDC, N_DIM], MMD)
            wk_s = pw3.tile([128, NDC, N_DIM], MMD)
            wv_s = pw3.tile([128, NDC, N_DIM], MMD)
            wf_s = pW.tile([128, NDC, N_DIM], MMD)
            hqv = hqT_d.rearrange("(c p) l -> p c l", p=128)
            hkv = hkT_d.rearrange("(c p) l -> p c l", p=128)
            # The first projection matmuls gate on c=0 of hq/hk: quarter those
            # so they land on four DMA queues each (one dma_start = one queue),
            # longest transfers first; later chunks are halved.
            for q in range(4):
                nc.sync.dma_start(out=hq_s[:, 0, q * 256:(q + 1) * 256],
                                  in_=hqv[:, 0, q * 256:(q + 1) * 256])
                nc.sync.dma_start(out=hk_s[:, 0, q * 256:(q + 1) * 256],
                                  in_=hkv[:, 0, q * 256:(q + 1) * 256])
            nc.sync.dma_start(out=wq_s[:, 0, :],
                              in_=wqT_d.rearrange("(c p) o -> p c o", p=128)[:, 0, :])
            nc.sync.dma_start(out=wk_s[:, 0, :],
                              in_=wkT_d.rearrange("(c p) o -> p c o", p=128)[:, 0, :])
            for c in range(1, NDC):
                nc.sync.dma_start(out=wq_s[:, c, :],
                                  in_=wqT_d.rearrange("(c p) o -> p c o", p=128)[:, c, :])
                for half in range(2):
                    nc.sync.dma_start(out=hq_s[:, c, half * 512:(half + 1) * 512],
                                      in_=hqv[:, c, half * 512:(half + 1) * 512])
                nc.sync.dma_start(out=wk_s[:, c, :],
                                  in_=wkT_d.rearrange("(c p) o -> p c o", p=128)[:, c, :])
                for half in range(2):
                    nc.sync.dma_start(out=hk_s[:, c, half * 512:(half + 1) * 512],
                                      in_=hkv[:, c, half * 512:(half + 1) * 512])
            for c in range(NDC):
                nc.sync.dma_start(out=wv_s[:, c, :],
                                  in_=wvT_d.rearrange("(c p) o -> p c o", p=128)[:, c, :])
            hvv = hvT_d.rearrange("(c p) l -> p c l", p=128)
            for n in range(2):
                for cc in range(2):
                    nc.sync.dma_start(
                        out=hv_s[:, cc * 2:(cc + 1) * 2, n * 512:(n + 1) * 512],
                        in_=hvv[:, cc * 2:(cc + 1) * 2, n * 512:(n + 1) * 512])
            for c in range(NDC):
                nc.sync.dma_start(out=wf_s[:, c, :],
                                  in_=wfT_d.rearrange("(c p) o -> p c o", p=128)[:, c, :])

            # ---- persistent tiles ----
            qt65 = [pQK.tile([128, L], MMD, name=f"qt65_{h}") for h in range(H)]
            kt65 = [pQK.tile([128, L], MMD, name=f"kt65_{h}") for h in range(H)]
            for h in range(H):
                nc.gpsimd.memset(kt65[h][64:65, :].bitcast(F32), 1.0)
                nc.gpsimd.memset(qt65[h][64:65, :].bitcast(F32), 0.0)

            v_s = pV.tile([128, NKC, VW], MMD)          # v[k, o] (+64 pad cols)
            nc.gpsimd.memset(v_s[:, :, N_DIM:VW].bitcast(F32), 0.0)
            res_sb = pRes.tile([128, NDC, L], MMD)      # res^T chunked by feature

            # sparsemax working set: double-buffered by head parity.
            # cs*/sf* are 16 wide with a permanently-zero half so cumsum /
            # suffix-sum shift-adds need no boundary copies.
            csA16 = [pSm.tile([128, NQT, 16], F32, name=f"csA16_{p}") for p in range(2)]
            csB16 = [pSm.tile([128, NQT, 16], F32, name=f"csB16_{p}") for p in range(2)]
            sfA16 = [pSm.tile([128, NQT, 16], F32, name=f"sfA16_{p}") for p in range(2)]
            sfB16 = [pSm.tile([128, NQT, 16], F32, name=f"sfB16_{p}") for p in range(2)]
            for p in range(2):
                nc.gpsimd.memset(csA16[p][:, :, 0:8], 0.0)
                nc.gpsimd.memset(csB16[p][:, :, 0:8], 0.0)
                nc.gpsimd.memset(sfA16[p][:, :, 8:16], 0.0)
                nc.gpsimd.memset(sfB16[p][:, :, 8:16], 0.0)
            tj = pSm.tile([128, NQT, 16], F32)
            tauPad = pSm.tile([128, 72], F32)
            nc.gpsimd.memset(tauPad[:, 8:72], 0.0)
            negTauT = pSm.tile([8, 128], MMD)

            # ---- stage 1a: QT / KT projections ----
            copy_flip = [0]

            def psum_copy(dst, src):
                # alternate ACT / DVE for PSUM evacuation
                if copy_flip[0] % 2 == 0:
                    nc.scalar.activation(dst, src, AF.Copy)
                else:
                    nc.vector.tensor_copy(dst, src)
                copy_flip[0] += 1

            def emit_proj(j, n, w_s, h_s, dst):
                pj = psA.tile([128, 512], F32, tag="a", name="projp")
                for c in range(NDC):
                    nc.tensor.matmul(
                        pj,
                        w_s[:, c, j * 128:(j + 1) * 128],
                        h_s[:, c, n * 512:(n + 1) * 512],
                        start=(c == 0), stop=(c == NDC - 1))
                psum_copy(dst[2 * j][0:64, n * 512:(n + 1) * 512], pj[0:64, :])
                psum_copy(dst[2 * j + 1][0:64, n * 512:(n + 1) * 512], pj[64:128, :])

            # per-head S-phase working tiles
            def head_tiles():
                C = pWk.tile([128, NQT, 16], F32, tag="C", name="C")
                negC = pWk.tile([128, NQT, 16], F32, tag="negC", name="negC")
                return C, negC

            def emit_S_qt(h, qt, C, negC):
                """S matmuls (K=65, full PE mode) + maxes for (head h, q-tile qt).
                M1 lands in csA16[h%2][:, qt, 8:16]; M2 for qt-1 is emitted by the
                caller (skewed so the GPSIMD negC has time to land)."""
                par = h % 2
                for kh in range(2):
                    s_ps = psA.tile([128, 512], F32, tag="a", name="s_ps")
                    nc.tensor.matmul(
                        s_ps,
                        qt65[h][0:65, qt * 128:(qt + 1) * 128],
                        kt65[h][0:65, kh * 512:(kh + 1) * 512],
                        start=True, stop=True)
                    nc.vector.max(out=C[:, qt, kh * 8:(kh + 1) * 8], in_=s_ps)
                nc.vector.max(out=csA16[par][:, qt, 8:16], in_=C[:, qt, :])
                nc.gpsimd.tensor_scalar(out=negC[:, qt, :], in0=C[:, qt, :],
                                        scalar1=-1.0, scalar2=None, op0=AT.mult)

            def emit_M2(h, qt, negC):
                nc.vector.max(out=sfA16[h % 2][:, qt, 0:8], in_=negC[:, qt, :])

            def emit_V_kc(kc):
                pv = psA.tile([128, 512], F32, tag="a", name="vp")
                for c in range(NDC):
                    nc.tensor.matmul(
                        pv,
                        hv_s[:, c, kc * 128:(kc + 1) * 128],
                        wv_s[:, c, :],
                        start=(c == 0), stop=(c == NDC - 1))
                nc.scalar.activation(v_s[:, kc, 0:N_DIM], pv, AF.Copy)

            def emit_tau_chain(h):
                """cumsum/suffix (GPSIMD shift-adds), candidates + reduce (DVE)."""
                par = h % 2
                cA, cB, sA, sB = csA16[par], csB16[par], sfA16[par], sfB16[par]
                # cumsum of M1 (data in [:, :, 8:16], zeros to the left)
                nc.gpsimd.tensor_tensor(out=cB[:, :, 8:16], in0=cA[:, :, 8:16],
                                        in1=cA[:, :, 7:15], op=AT.add)
                nc.gpsimd.tensor_tensor(out=cA[:, :, 8:16], in0=cB[:, :, 8:16],
                                        in1=cB[:, :, 6:14], op=AT.add)
                nc.gpsimd.tensor_tensor(out=cB[:, :, 8:16], in0=cA[:, :, 8:16],
                                        in1=cA[:, :, 4:12], op=AT.add)
                # suffix sums of M2 (data in [:, :, 0:8], zeros to the right)
                nc.gpsimd.tensor_tensor(out=sB[:, :, 0:8], in0=sA[:, :, 0:8],
                                        in1=sA[:, :, 1:9], op=AT.add)
                nc.gpsimd.tensor_tensor(out=sA[:, :, 0:8], in0=sB[:, :, 0:8],
                                        in1=sB[:, :, 2:10], op=AT.add)
                nc.gpsimd.tensor_tensor(out=sB[:, :, 0:8], in0=sA[:, :, 0:8],
                                        in1=sA[:, :, 4:12], op=AT.add)
                # tj[0:8]  = (cs_j - 1) * (1/j)
                nc.vector.scalar_tensor_tensor(
                    out=tj[:, :, 0:8], in0=cB[:, :, 8:16], scalar=1.0,
                    in1=recj[:, 0:8].unsqueeze(1).to_broadcast([128, NQT, 8]),
                    op0=AT.subtract, op1=AT.mult)
                # tj[8:16] = (cs_8 - r_p - 1) * 1/(16-p)
                nc.vector.tensor_tensor(
                    out=tj[:, :, 8:16],
                    in0=cB[:, :, 15:16].to_broadcast([128, NQT, 8]),
                    in1=sB[:, :, 0:8], op=AT.subtract)
                nc.vector.scalar_tensor_tensor(
                    out=tj[:, :, 8:16], in0=tj[:, :, 8:16], scalar=1.0,
                    in1=recj[:, 16:24].unsqueeze(1).to_broadcast([128, NQT, 8]),
                    op0=AT.subtract, op1=AT.mult)
                nc.vector.tensor_reduce(out=tauPad[:, 0:8], in_=tj,
                                        axis=AX.X, op=AT.max)

            def emit_tau_plumb(h):
                """PE transpose (128x128 mode via 72-col pad) -> negate -> row 64."""
                tauT_ps = psT.tile([72, 128], F32, tag="tauT", name="tauT")
                nc.tensor.transpose(tauT_ps, tauPad, identity)
                nc.scalar.activation(negTauT, tauT_ps[0:8, :], AF.Copy,
                                     bias=0.0, scale=-1.0)
                nc.scalar.dma_start(out=qt65[h][64:65, :], in_=negTauT)

            def emit_ST_kc(h, kc):
                """S^T - tau (K=65) -> relu -> alpha^T.  Returns alphaT tile."""
                alphaT = pA.tile([128, L], MMD, tag="alphaT", name="alphaT")
                for qh in range(2):
                    st_ps = psB.tile([128, 512], F32, tag="b", name="st_ps")
                    nc.tensor.matmul(
                        st_ps,
                        kt65[h][0:65, kc * 128:(kc + 1) * 128],
                        qt65[h][0:65, qh * 512:(qh + 1) * 512],
                        start=True, stop=True)
                    nc.scalar.activation(alphaT[:, qh * 512:(qh + 1) * 512], st_ps, AF.Relu)
                return alphaT

            def emit_AV_kc(h, kc, alphaT, res_ps):
                """AV accumulate; 128-col stationary (head h cols 0:64 valid)."""
                for qh in range(2):
                    nc.tensor.matmul(
                        res_ps[:, qh * 512:(qh + 1) * 512],
                        v_s[:, kc, h * 64:h * 64 + 128],
                        alphaT[:, qh * 512:(qh + 1) * 512],
                        start=(kc == 0), stop=(kc == NKC - 1))

            # ---- S-unit emitter with skewed M2 (gives GPSIMD negC time) ----
            pending_m2 = [None]

            def emit_S_unit(h, qt, C, negC):
                if pending_m2[0] is not None:
                    emit_M2(*pending_m2[0])
                emit_S_qt(h, qt, C, negC)
                pending_m2[0] = (h, qt, negC)

            def flush_M2():
                if pending_m2[0] is not None:
                    emit_M2(*pending_m2[0])
                    pending_m2[0] = None

            # ---- prologue: projections with head-0/1 S phases interleaved ----
            # (2-deep pipeline: S(h) runs two slots ahead of B(h) so the tau
            # chain/plumb latency is fully hidden.)
            tiles = {0: head_tiles(), 1: head_tiles()}
            squeue = [(0, q) for q in range(NQT)] + [(1, q) for q in range(NQT)]
            sq = 0
            for n in range(2):
                emit_proj(0, n, wq_s, hq_s, qt65)
                emit_proj(0, n, wk_s, hk_s, kt65)
            for j in range(1, NDC):
                for n in range(2):
                    emit_proj(j, n, wq_s, hq_s, qt65)
                    emit_proj(j, n, wk_s, hk_s, kt65)
                for _ in range(2):
                    hh, qq = squeue[sq]; sq += 1
                    emit_S_unit(hh, qq, *tiles[hh])
            for kc in range(NKC):
                hh, qq = squeue[sq]; sq += 1
                emit_S_unit(hh, qq, *tiles[hh])
                emit_V_kc(kc)
                if kc == 2:
                    emit_tau_chain(0)
                    emit_tau_plumb(0)
            while sq < len(squeue):
                hh, qq = squeue[sq]; sq += 1
                emit_S_unit(hh, qq, *tiles[hh])
            flush_M2()

            # ---- main pipelined loop over heads ----
            for h in range(H):
                res_ps = psR.tile([128, L], F32, tag="res", name="res_ps")
                if h + 2 < H:
                    tiles[h + 2] = head_tiles()
                prev = None  # (kc, alphaT) awaiting AV
                for i in range(NQT):
                    if h + 2 < H:
                        emit_S_unit(h + 2, i, *tiles[h + 2])
                    alphaT = emit_ST_kc(h, i)
                    if prev is not None:
                        emit_AV_kc(h, prev[0], prev[1], res_ps)
                    prev = (i, alphaT)
                    if i == 1 and h + 1 < H:
                        # chain is DVE/GPSIMD-only and safe mid-slot; the plumb
                        # contains a PE transpose which must not land inside the
                        # open AV PSUM accumulation group, so it waits for the
                        # group's stop below
                        emit_tau_chain(h + 1)
                emit_AV_kc(h, prev[0], prev[1], res_ps)
                if h + 1 < H:
                    emit_tau_plumb(h + 1)
                flush_M2()
                half = 64 * (h % 2)
                nc.scalar.activation(res_sb[half:half + 64, h // 2, :],
                                     res_ps[0:64, :], AF.Copy)

            # ---- stage 3: final projection + bias ----
            for m in range(NDC):
                for n in range(2):
                    po = psB.tile([128, 512], F32, tag="b", name="po")
                    for c in range(NDC):
                        nc.tensor.matmul(
                            po,
                            wf_s[:, c, m * 128:(m + 1) * 128],
                            res_sb[:, c, n * 512:(n + 1) * 512],
                            start=(c == 0), stop=(c == NDC - 1))
                    ot = pOut.tile([128, 512], F32, tag="ot", name="ot")
                    nc.vector.tensor_scalar(out=ot, in0=po,
                                            scalar1=bf2_s[:, m:m + 1], scalar2=None,
                                            op0=AT.add)
                    for q in range(4):
                        nc.sync.dma_start(
                            out=outT_d.rearrange("(m p) l -> p m l", p=128)[
                                :, m, n * 512 + q * 128:n * 512 + (q + 1) * 128],
                            in_=ot[:, q * 128:(q + 1) * 128])

    nc.compile()
    return nc


def _round_f32r(x):
    """Round fp32 array to the fp32r grid (11-bit mantissa, round-to-nearest)."""
    if not MM_DTYPE_F32R:
        return np.ascontiguousarray(x, dtype=np.float32)
    v = np.ascontiguousarray(x, dtype=np.float32).view(np.uint32)
    r = ((v.astype(np.uint64) + 0x800) & 0xFFFFF000).astype(np.uint32)
    return r.view(np.float32)


def _prep_inputs(h_q, h_k, h_v, Wq, Wk, Wv, bv, Wf, bf):
    f32 = np.float32
    wqT = _round_f32r((np.asarray(Wq, f32) / TEMPERATURE).T)
    wkT = _round_f32r(np.asarray(Wk, f32).T)
    wvT = _round_f32r(np.asarray(Wv, f32).T)
    wfT = _round_f32r(np.asarray(Wf, f32).T)
    bf2 = (np.asarray(Wf, np.float64) @ np.asarray(bv, np.float64)
           + np.asarray(bf, np.float64)).astype(f32)
    rec = np.zeros(32, dtype=f32)
    rec[0:16] = (1.0 / np.arange(1, 17, dtype=np.float64)).astype(f32)
    rec[16:24] = (1.0 / np.arange(16, 8, -1, dtype=np.float64)).astype(f32)
    recj = np.ascontiguousarray(np.broadcast_to(rec, (128, 32)))
    shared = {"wqT": wqT, "wkT": wkT, "wvT": wvT, "wfT": wfT, "bf2": bf2, "recj": recj}
    in_maps = []
    for b in range(BS):
        m = dict(shared)
        m["hqT"] = _round_f32r(np.asarray(h_q[b], f32).T)
        m["hkT"] = _round_f32r(np.asarray(h_k[b], f32).T)
        m["hvT"] = _round_f32r(np.asarray(h_v[b], f32).T)
        in_maps.append(m)
    return in_maps


def kernel(h_q, h_k, h_v, Wq, Wk, Wv, bv, Wf, bf):
    from concourse.bass_utils import run_bass_kernel_spmd

    if "nc" not in _COMPILED:
        _COMPILED["nc"] = _build_nc()
    nc = _COMPILED["nc"]

    in_maps = _prep_inputs(h_q, h_k, h_v, Wq, Wk, Wv, bv, Wf, bf)
    res = run_bass_kernel_spmd(nc, in_maps, core_ids=list(range(BS)))
    out = np.empty((BS, L, N_DIM), dtype=np.float32)
    for b in range(BS):
        out[b] = res.results[b]["outT"].T
    return out


if __name__ == "__main__":
    rng = np.random.default_rng(0)
    d = N_DIM
    s = 1.0 / np.sqrt(d)
    ins = {
        "h_q": rng.standard_normal((BS, L, d), dtype=np.float32),
        "h_k": rng.standard_normal((BS, L, d), dtype=np.float32),
        "h_v": rng.standard_normal((BS, L, d), dtype=np.float32),
        "Wq": rng.standard_normal((d, d), dtype=np.float32) * s,
        "Wk": rng.standard_normal((d, d), dtype=np.float32) * s,
        "Wv": rng.standard_normal((d, d), dtype=np.float32) * s,
        "bv": rng.standard_normal((d,), dtype=np.float32) * s,
        "Wf": rng.standard_normal((d, d), dtype=np.float32) * s,
        "bf": rng.standard_normal((d,), dtype=np.float32) * s,
    }
    out = kernel(**ins)
    print("kernel ran, out shape", out.shape)


# revision 21
# speedup vs baseline: 1.0147x; 1.0147x over previous
"""Trainium2 Bass kernel: multi-head attention with sparsemax (sparse attention).

Problem: nn_MultiHeadAttention_24309514895753
  bs=8, L=1024, d=512, H=8 heads, head dim D=64, fp32.
  out = sparsemax((h_q Wq^T / sqrt(D)) (h_k Wk^T)^T) (h_v Wv^T + bv) Wf^T + bf

Sharding: data-parallel over batch (8 cores, core b owns batch element b).
No collectives needed.

Per-core algorithm (same math as the baseline, restructured for overlap):
  1. Projections on PE in transposed layout: QT[o,l] (pre-scaled by 1/temp),
     KT[o,l], V[l,o].  Bias bv folded into the final bias on the host.
     Input DMAs are chunked; head-0 S matmuls interleave with the
     projection matmuls.
  2. Per head h: S = Q_h K_h^T into PSUM per (q-tile, k-half); DVE max8 ->
     top-8 per 512-half, top-8 (M1) / negated top-8 (M2) of the 16
     candidates.  tau = max_j (cumsum_j - 1)/j over the sorted top-16:
     cumsum/suffix sums on GPSIMD via zero-padded shift-adds, candidate
     formation + reduce on DVE.  tau -> PE transpose -> negate -> row 64
     of the 65-row QT tile (KT row 64 = ones).
  3. S^T - tau (K=65) -> Relu (ACT) -> alpha^T; AV accumulates res^T.
  4. Final projection out^T = Wf res + bias, DMA out^T; host transposes.

All matmuls (S, S^T, AV, projections, tau transpose) are shaped to run in
the PE's full 128x128 tiling mode -- S contracts over K=65 (row 64 of QT
is zero during the S phase, so it adds exact 0), AV uses a 128-column
stationary operand spanning two heads' V (the lower 64 output rows are
ignored), and the tau transpose input is padded to 72 columns.  Mixed
tile modes force PE array drains on every mode switch (~2x matmul cost).

Pipelining (2-deep): head h+2's S matmuls interleave with head h's
S^T/AV matmuls in PE issue order, so head h+1's tau chain (computed from
S(h+1) during slot h-1) is never on the PE critical path.  AV lags S^T by
one k-chunk so Relu latency is hidden.  The tau row DMA is issued from
the scalar engine (the sync queue is busy issuing input-DMA doorbells
early on), and input/output DMAs are chunked/split so transfers spread
across DMA queues and overlap compute.

Matmul dtype: float32r (11-bit mantissa, 4x fp32 rate).  Inputs pre-rounded
to the fp32r grid on the host so S and S^T are bit-consistent.
"""

import numpy as np

N_HEADS = 8
N_DIM = 512
ATTN_DIM = 64
TEMPERATURE = ATTN_DIM ** 0.5
BS = 8
L = 1024

MM_DTYPE_F32R = True

_COMPILED = {}


def _build_nc():
    import concourse.bacc as bacc
    import concourse.mybir as mybir
    import concourse.tile as tile
    from concourse.masks import make_identity

    F32 = mybir.dt.float32
    MMD = mybir.dt.float32r if MM_DTYPE_F32R else F32
    AT = mybir.AluOpType
    AF = mybir.ActivationFunctionType
    AX = mybir.AxisListType

    nc = bacc.Bacc("TRN2", target_bir_lowering=False, debug=False, num_devices=8)

    hqT_d = nc.dram_tensor("hqT", [N_DIM, L], MMD, kind="ExternalInput").ap()
    hkT_d = nc.dram_tensor("hkT", [N_DIM, L], MMD, kind="ExternalInput").ap()
    hvT_d = nc.dram_tensor("hvT", [N_DIM, L], MMD, kind="ExternalInput").ap()
    wqT_d = nc.dram_tensor("wqT", [N_DIM, N_DIM], MMD, kind="ExternalInput").ap()
    wkT_d = nc.dram_tensor("wkT", [N_DIM, N_DIM], MMD, kind="ExternalInput").ap()
    wvT_d = nc.dram_tensor("wvT", [N_DIM, N_DIM], MMD, kind="ExternalInput").ap()
    wfT_d = nc.dram_tensor("wfT", [N_DIM, N_DIM], MMD, kind="ExternalInput").ap()
    bf2_d = nc.dram_tensor("bf2", [N_DIM], F32, kind="ExternalInput").ap()
    rec_d = nc.dram_tensor("recj", [128, 32], F32, kind="ExternalInput").ap()
    outT_d = nc.dram_tensor("outT", [N_DIM, L], F32, kind="ExternalOutput").ap()

    H = N_HEADS
    NQT = L // 128          # 8 q tiles per head
    NKC = L // 128          # 8 k chunks per head
    NDC = N_DIM // 128      # 4 feature chunks
    VW = N_DIM + 64         # v_s free width: 64 zero-pad cols for head 7's AV

    with tile.TileContext(nc) as tc:
        with tc.tile_pool(name="pW", bufs=1) as pW, \
             tc.tile_pool(name="pQK", bufs=1) as pQK, \
             tc.tile_pool(name="pV", bufs=1) as pV, \
             tc.tile_pool(name="pRes", bufs=1) as pRes, \
             tc.tile_pool(name="pOut", bufs=4) as pOut, \
             tc.tile_pool(name="pSm", bufs=1) as pSm, \
             tc.tile_pool(name="pWk", bufs=2) as pWk, \
             tc.tile_pool(name="pA", bufs=3) as pA, \
             tc.tile_pool(name="pIn", bufs=1) as pIn, \
             tc.tile_pool(name="pw3", bufs=1) as pw3, \
             tc.tile_pool(name="psA", bufs=2, space="PSUM") as psA, \
             tc.tile_pool(name="psB", bufs=3, space="PSUM") as psB, \
             tc.tile_pool(name="psR", bufs=1, space="PSUM") as psR, \
             tc.tile_pool(name="psT", bufs=1, space="PSUM") as psT:

            # ---- small constants first ----
            recj = pW.tile([128, 32], F32)
            nc.sync.dma_start(out=recj, in_=rec_d)
            bf2_s = pW.tile([128, NDC], F32)
            nc.sync.dma_start(out=bf2_s, in_=bf2_d.rearrange("(m p) -> p m", p=128))
            identity = pW.tile([128, 128], F32)
            make_identity(nc, identity)

            # ---- input staging (all DMAs issued up front, chunked) ----
            hq_s = pIn.tile([128, NDC, L], MMD)
            hk_s = pIn.tile([128, NDC, L], MMD)
            hv_s = pIn.tile([128, NDC, L], MMD)
            wq_s = pw3.tile([128, NDC, N_DIM], MMD)
            wk_s = pw3.tile([128, NDC, N_DIM], MMD)
            wv_s = pw3.tile([128, NDC, N_DIM], MMD)
            wf_s = pW.tile([128, NDC, N_DIM], MMD)
            hqv = hqT_d.rearrange("(c p) l -> p c l", p=128)
            hkv = hkT_d.rearrange("(c p) l -> p c l", p=128)
            # c=0 feeds the first projection matmuls: halve the big activation
            # chunks so they land on two DMA queues each, longest first
            for half in range(2):
                nc.sync.dma_start(out=hq_s[:, 0, half * 512:(half + 1) * 512],
                                  in_=hqv[:, 0, half * 512:(half + 1) * 512])
                nc.sync.dma_start(out=hk_s[:, 0, half * 512:(half + 1) * 512],
                                  in_=hkv[:, 0, half * 512:(half + 1) * 512])
            nc.sync.dma_start(out=wq_s[:, 0, :],
                              in_=wqT_d.rearrange("(c p) o -> p c o", p=128)[:, 0, :])
            nc.sync.dma_start(out=wk_s[:, 0, :],
                              in_=wkT_d.rearrange("(c p) o -> p c o", p=128)[:, 0, :])
            for c in range(1, NDC):
                nc.sync.dma_start(out=wq_s[:, c, :],
                                  in_=wqT_d.rearrange("(c p) o -> p c o", p=128)[:, c, :])
                nc.sync.dma_start(out=hq_s[:, c, :], in_=hqv[:, c, :])
                nc.sync.dma_start(out=wk_s[:, c, :],
                                  in_=wkT_d.rearrange("(c p) o -> p c o", p=128)[:, c, :])
                nc.sync.dma_start(out=hk_s[:, c, :], in_=hkv[:, c, :])
            for c in range(NDC):
                nc.sync.dma_start(out=wv_s[:, c, :],
                                  in_=wvT_d.rearrange("(c p) o -> p c o", p=128)[:, c, :])
            for n in range(2):
                nc.sync.dma_start(
                    out=hv_s[:, :, n * 512:(n + 1) * 512],
                    in_=hvT_d.rearrange("(c p) l -> p c l", p=128)[:, :, n * 512:(n + 1) * 512])
            for c in range(NDC):
                nc.sync.dma_start(out=wf_s[:, c, :],
                                  in_=wfT_d.rearrange("(c p) o -> p c o", p=128)[:, c, :])

            # ---- persistent tiles ----
            qt65 = [pQK.tile([128, L], MMD, name=f"qt65_{h}") for h in range(H)]
            kt65 = [pQK.tile([128, L], MMD, name=f"kt65_{h}") for h in range(H)]
            for h in range(H):
                nc.gpsimd.memset(kt65[h][64:65, :].bitcast(F32), 1.0)
                nc.gpsimd.memset(qt65[h][64:65, :].bitcast(F32), 0.0)

            v_s = pV.tile([128, NKC, VW], MMD)          # v[k, o] (+64 pad cols)
            nc.gpsimd.memset(v_s[:, :, N_DIM:VW].bitcast(F32), 0.0)
            res_sb = pRes.tile([128, NDC, L], MMD)      # res^T chunked by feature

            # sparsemax working set: double-buffered by head parity.
            # cs*/sf* are 16 wide with a permanently-zero half so cumsum /
            # suffix-sum shift-adds need no boundary copies.
            csA16 = [pSm.tile([128, NQT, 16], F32, name=f"csA16_{p}") for p in range(2)]
            csB16 = [pSm.tile([128, NQT, 16], F32, name=f"csB16_{p}") for p in range(2)]
            sfA16 = [pSm.tile([128, NQT, 16], F32, name=f"sfA16_{p}") for p in range(2)]
            sfB16 = [pSm.tile([128, NQT, 16], F32, name=f"sfB16_{p}") for p in range(2)]
            for p in range(2):
                nc.gpsimd.memset(csA16[p][:, :, 0:8], 0.0)
                nc.gpsimd.memset(csB16[p][:, :, 0:8], 0.0)
                nc.gpsimd.memset(sfA16[p][:, :, 8:16], 0.0)
                nc.gpsimd.memset(sfB16[p][:, :, 8:16], 0.0)
            tj = pSm.tile([128, NQT, 16], F32)
            tauPad = pSm.tile([128, 72], F32)
            nc.gpsimd.memset(tauPad[:, 8:72], 0.0)
            negTauT = pSm.tile([8, 128], MMD)

            # ---- stage 1a: QT / KT projections ----
            copy_flip = [0]

            def psum_copy(dst, src):
                # alternate ACT / DVE for PSUM evacuation
                if copy_flip[0] % 2 == 0:
                    nc.scalar.activation(dst, src, AF.Copy)
                else:
                    nc.vector.tensor_copy(dst, src)
                copy_flip[0] += 1

            def emit_proj(j, n, w_s, h_s, dst):
                pj = psA.tile([128, 512], F32, tag="a", name="projp")
                for c in range(NDC):
                    nc.tensor.matmul(
                        pj,
                        w_s[:, c, j * 128:(j + 1) * 128],
                        h_s[:, c, n * 512:(n + 1) * 512],
                        start=(c == 0), stop=(c == NDC - 1))
                psum_copy(dst[2 * j][0:64, n * 512:(n + 1) * 512], pj[0:64, :])
                psum_copy(dst[2 * j + 1][0:64, n * 512:(n + 1) * 512], pj[64:128, :])

            # per-head S-phase working tiles
            def head_tiles():
                C = pWk.tile([128, NQT, 16], F32, tag="C", name="C")
                negC = pWk.tile([128, NQT, 16], F32, tag="negC", name="negC")
                return C, negC

            def emit_S_qt(h, qt, C, negC):
                """S matmuls (K=65, full PE mode) + maxes for (head h, q-tile qt).
                M1 lands in csA16[h%2][:, qt, 8:16]; M2 for qt-1 is emitted by the
                caller (skewed so the GPSIMD negC has time to land)."""
                par = h % 2
                for kh in range(2):
                    s_ps = psA.tile([128, 512], F32, tag="a", name="s_ps")
                    nc.tensor.matmul(
                        s_ps,
                        qt65[h][0:65, qt * 128:(qt + 1) * 128],
                        kt65[h][0:65, kh * 512:(kh + 1) * 512],
                        start=True, stop=True)
                    nc.vector.max(out=C[:, qt, kh * 8:(kh + 1) * 8], in_=s_ps)
                nc.vector.max(out=csA16[par][:, qt, 8:16], in_=C[:, qt, :])
                nc.gpsimd.tensor_scalar(out=negC[:, qt, :], in0=C[:, qt, :],
                                        scalar1=-1.0, scalar2=None, op0=AT.mult)

            def emit_M2(h, qt, negC):
                nc.vector.max(out=sfA16[h % 2][:, qt, 0:8], in_=negC[:, qt, :])

            def emit_V_kc(kc):
                pv = psA.tile([128, 512], F32, tag="a", name="vp")
                for c in range(NDC):
                    nc.tensor.matmul(
                        pv,
                        hv_s[:, c, kc * 128:(kc + 1) * 128],
                        wv_s[:, c, :],
                        start=(c == 0), stop=(c == NDC - 1))
                nc.scalar.activation(v_s[:, kc, 0:N_DIM], pv, AF.Copy)

            def emit_tau_chain(h):
                """cumsum/suffix (GPSIMD shift-adds), candidates + reduce (DVE)."""
                par = h % 2
                cA, cB, sA, sB = csA16[par], csB16[par], sfA16[par], sfB16[par]
                # cumsum of M1 (data in [:, :, 8:16], zeros to the left)
                nc.gpsimd.tensor_tensor(out=cB[:, :, 8:16], in0=cA[:, :, 8:16],
                                        in1=cA[:, :, 7:15], op=AT.add)
                nc.gpsimd.tensor_tensor(out=cA[:, :, 8:16], in0=cB[:, :, 8:16],
                                        in1=cB[:, :, 6:14], op=AT.add)
                nc.gpsimd.tensor_tensor(out=cB[:, :, 8:16], in0=cA[:, :, 8:16],
                                        in1=cA[:, :, 4:12], op=AT.add)
                # suffix sums of M2 (data in [:, :, 0:8], zeros to the right)
                nc.gpsimd.tensor_tensor(out=sB[:, :, 0:8], in0=sA[:, :, 0:8],
                                        in1=sA[:, :, 1:9], op=AT.add)
                nc.gpsimd.tensor_tensor(out=sA[:, :, 0:8], in0=sB[:, :, 0:8],
                                        in1=sB[:, :, 2:10], op=AT.add)
                nc.gpsimd.tensor_tensor(out=sB[:, :, 0:8], in0=sA[:, :, 0:8],
                                        in1=sA[:, :, 4:12], op=AT.add)
                # tj[0:8]  = (cs_j - 1) * (1/j)
                nc.vector.scalar_tensor_tensor(
                    out=tj[:, :, 0:8], in0=cB[:, :, 8:16], scalar=1.0,
                    in1=recj[:, 0:8].unsqueeze(1).to_broadcast([128, NQT, 8]),
                    op0=AT.subtract, op1=AT.mult)
                # tj[8:16] = (cs_8 - r_p - 1) * 1/(16-p)
                nc.vector.tensor_tensor(
                    out=tj[:, :, 8:16],
                    in0=cB[:, :, 15:16].to_broadcast([128, NQT, 8]),
                    in1=sB[:, :, 0:8], op=AT.subtract)
                nc.vector.scalar_tensor_tensor(
                    out=tj[:, :, 8:16], in0=tj[:, :, 8:16], scalar=1.0,
                    in1=recj[:, 16:24].unsqueeze(1).to_broadcast([128, NQT, 8]),
                    op0=AT.subtract, op1=AT.mult)
                nc.vector.tensor_reduce(out=tauPad[:, 0:8], in_=tj,
                                        axis=AX.X, op=AT.max)

            def emit_tau_plumb(h):
                """PE transpose (128x128 mode via 72-col pad) -> negate -> row 64."""
                tauT_ps = psT.tile([72, 128], F32, tag="tauT", name="tauT")
                nc.tensor.transpose(tauT_ps, tauPad, identity)
                nc.scalar.activation(negTauT, tauT_ps[0:8, :], AF.Copy,
                                     bias=0.0, scale=-1.0)
                nc.scalar.dma_start(out=qt65[h][64:65, :], in_=negTauT)

            def emit_ST_kc(h, kc):
                """S^T - tau (K=65) -> relu -> alpha^T.  Returns alphaT tile."""
                alphaT = pA.tile([128, L], MMD, tag="alphaT", name="alphaT")
                for qh in range(2):
                    st_ps = psB.tile([128, 512], F32, tag="b", name="st_ps")
                    nc.tensor.matmul(
                        st_ps,
                        kt65[h][0:65, kc * 128:(kc + 1) * 128],
                        qt65[h][0:65, qh * 512:(qh + 1) * 512],
                        start=True, stop=True)
                    nc.scalar.activation(alphaT[:, qh * 512:(qh + 1) * 512], st_ps, AF.Relu)
                return alphaT

            def emit_AV_kc(h, kc, alphaT, res_ps):
                """AV accumulate; 128-col stationary (head h cols 0:64 valid)."""
                for qh in range(2):
                    nc.tensor.matmul(
                        res_ps[:, qh * 512:(qh + 1) * 512],
                        v_s[:, kc, h * 64:h * 64 + 128],
                        alphaT[:, qh * 512:(qh + 1) * 512],
                        start=(kc == 0), stop=(kc == NKC - 1))

            # ---- S-unit emitter with skewed M2 (gives GPSIMD negC time) ----
            pending_m2 = [None]

            def emit_S_unit(h, qt, C, negC):
                if pending_m2[0] is not None:
                    emit_M2(*pending_m2[0])
                emit_S_qt(h, qt, C, negC)
                pending_m2[0] = (h, qt, negC)

            def flush_M2():
                if pending_m2[0] is not None:
                    emit_M2(*pending_m2[0])
                    pending_m2[0] = None

            # ---- prologue: projections with head-0/1 S phases interleaved ----
            # (2-deep pipeline: S(h) runs two slots ahead of B(h) so the tau
            # chain/plumb latency is fully hidden.)
            tiles = {0: head_tiles(), 1: head_tiles()}
            squeue = [(0, q) for q in range(NQT)] + [(1, q) for q in range(NQT)]
            sq = 0
            for n in range(2):
                emit_proj(0, n, wq_s, hq_s, qt65)
                emit_proj(0, n, wk_s, hk_s, kt65)
            for j in range(1, NDC):
                for n in range(2):
                    emit_proj(j, n, wq_s, hq_s, qt65)
                    emit_proj(j, n, wk_s, hk_s, kt65)
                for _ in range(2):
                    hh, qq = squeue[sq]; sq += 1
                    emit_S_unit(hh, qq, *tiles[hh])
            for kc in range(NKC):
                hh, qq = squeue[sq]; sq += 1
                emit_S_unit(hh, qq, *tiles[hh])
                emit_V_kc(kc)
                if kc == 2:
                    emit_tau_chain(0)
                    emit_tau_plumb(0)
            while sq < len(squeue):
                hh, qq = squeue[sq]; sq += 1
                emit_S_unit(hh, qq, *tiles[hh])
            flush_M2()

            # ---- main pipelined loop over heads ----
            for h in range(H):
                res_ps = psR.tile([128, L], F32, tag="res", name="res_ps")
                if h + 2 < H:
                    tiles[h + 2] = head_tiles()
                prev = None  # (kc, alphaT) awaiting AV
                for i in range(NQT):
                    if h + 2 < H:
                        emit_S_unit(h + 2, i, *tiles[h + 2])
                    alphaT = emit_ST_kc(h, i)
                    if prev is not None:
                        emit_AV_kc(h, prev[0], prev[1], res_ps)
                    prev = (i, alphaT)
                    if i == 1 and h + 1 < H:
                        # chain is DVE/GPSIMD-only and safe mid-slot; the plumb
                        # contains a PE transpose which must not land inside the
                        # open AV PSUM accumulation group, so it waits for the
                        # group's stop below
                        emit_tau_chain(h + 1)
                emit_AV_kc(h, prev[0], prev[1], res_ps)
                if h + 1 < H:
                    emit_tau_plumb(h + 1)
                flush_M2()
                half = 64 * (h % 2)
                nc.scalar.activation(res_sb[half:half + 64, h // 2, :],
                                     res_ps[0:64, :], AF.Copy)

            # ---- stage 3: final projection + bias ----
            for m in range(NDC):
                for n in range(2):
                    po = psB.tile([128, 512], F32, tag="b", name="po")
                    for c in range(NDC):
                        nc.tensor.matmul(
                            po,
                            wf_s[:, c, m * 128:(m + 1) * 128],
                            res_sb[:, c, n * 512:(n + 1) * 512],
                            start=(c == 0), stop=(c == NDC - 1))
                    ot = pOut.tile([128, 512], F32, tag="ot", name="ot")
                    nc.vector.tensor_scalar(out=ot, in0=po,
                                            scalar1=bf2_s[:, m:m + 1], scalar2=None,
                                            op0=AT.add)
                    for q in range(4):
                        nc.sync.dma_start(
                            out=outT_d.rearrange("(m p) l -> p m l", p=128)[
                                :, m, n * 512 + q * 128:n * 512 + (q + 1) * 128],
                            in_=ot[:, q * 128:(q + 1) * 128])

    nc.compile()
    return nc


def _round_f32r(x):
    """Round fp32 array to the fp32r grid (11-bit mantissa, round-to-nearest)."""
    if not MM_DTYPE_F32R:
        return np.ascontiguousarray(x, dtype=np.float32)
    v = np.ascontiguousarray(x, dtype=np.float32).view(np.uint32)
    r = ((v.astype(np.uint64) + 0x800) & 0xFFFFF000).astype(np.uint32)
    return r.view(np.float32)


def _prep_inputs(h_q, h_k, h_v, Wq, Wk, Wv, bv, Wf, bf):
    f32 = np.float32
    wqT = _round_f32r((np.asarray(Wq, f32) / TEMPERATURE).T)
    wkT = _round_f32r(np.asarray(Wk, f32).T)
    wvT = _round_f32r(np.asarray(Wv, f32).T)
    wfT = _round_f32r(np.asarray(Wf, f32).T)
    bf2 = (np.asarray(Wf, np.float64) @ np.asarray(bv, np.float64)
           + np.asarray(bf, np.float64)).astype(f32)
    rec = np.zeros(32, dtype=f32)
    rec[0:16] = (1.0 / np.arange(1, 17, dtype=np.float64)).astype(f32)
    rec[16:24] = (1.0 / np.arange(16, 8, -1, dtype=np.float64)).astype(f32)
    recj = np.ascontiguousarray(np.broadcast_to(rec, (128, 32)))
    shared = {"wqT": wqT, "wkT": wkT, "wvT": wvT, "wfT": wfT, "bf2": bf2, "recj": recj}
    in_maps = []
    for b in range(BS):
        m = dict(shared)
        m["hqT"] = _round_f32r(np.asarray(h_q[b], f32).T)
        m["hkT"] = _round_f32r(np.asarray(h_k[b], f32).T)
        m["hvT"] = _round_f32r(np.asarray(h_v[b], f32).T)
        in_maps.append(m)
    return in_maps


def kernel(h_q, h_k, h_v, Wq, Wk, Wv, bv, Wf, bf):
    from concourse.bass_utils import run_bass_kernel_spmd

    if "nc" not in _COMPILED:
        _COMPILED["nc"] = _build_nc()
    nc = _COMPILED["nc"]

    in_maps = _prep_inputs(h_q, h_k, h_v, Wq, Wk, Wv, bv, Wf, bf)
    res = run_bass_kernel_spmd(nc, in_maps, core_ids=list(range(BS)))
    out = np.empty((BS, L, N_DIM), dtype=np.float32)
    for b in range(BS):
        out[b] = res.results[b]["outT"].T
    return out


if __name__ == "__main__":
    rng = np.random.default_rng(0)
    d = N_DIM
    s = 1.0 / np.sqrt(d)
    ins = {
        "h_q": rng.standard_normal((BS, L, d), dtype=np.float32),
        "h_k": rng.standard_normal((BS, L, d), dtype=np.float32),
        "h_v": rng.standard_normal((BS, L, d), dtype=np.float32),
        "Wq": rng.standard_normal((d, d), dtype=np.float32) * s,
        "Wk": rng.standard_normal((d, d), dtype=np.float32) * s,
        "Wv": rng.standard_normal((d, d), dtype=np.float32) * s,
        "bv": rng.standard_normal((d,), dtype=np.float32) * s,
        "Wf": rng.standard_normal((d, d), dtype=np.float32) * s,
        "bf": rng.standard_normal((d,), dtype=np.float32) * s,
    }
    out = kernel(**ins)
    print("kernel ran, out shape", out.shape)


# revision 22
# speedup vs baseline: 1.1236x; 1.1073x over previous
"""Trainium2 Bass kernel: multi-head attention with sparsemax (sparse attention).

Problem: nn_MultiHeadAttention_24309514895753
  bs=8, L=1024, d=512, H=8 heads, head dim D=64, fp32.
  out = sparsemax((h_q Wq^T / sqrt(D)) (h_k Wk^T)^T) (h_v Wv^T + bv) Wf^T + bf

Sharding: data-parallel over batch (8 cores, core b owns batch element b).
No collectives needed.

Per-core algorithm (same math as the baseline, restructured for overlap):
  1. Projections on PE in transposed layout: QT[o,l] (pre-scaled by 1/temp),
     KT[o,l], V[l,o].  Bias bv folded into the final bias on the host.
     Input DMAs are chunked; head-0 S matmuls interleave with the
     projection matmuls.
  2. Per head h: S = Q_h K_h^T into PSUM per (q-tile, k-half); DVE max8 ->
     top-8 per 512-half, top-8 (M1) / negated top-8 (M2) of the 16
     candidates.  tau = max_j (cumsum_j - 1)/j over the sorted top-16:
     cumsum/suffix sums on GPSIMD via zero-padded shift-adds, candidate
     formation + reduce on DVE.  tau -> PE transpose -> negate -> row 64
     of the 65-row QT tile (KT row 64 = ones).
  3. S^T - tau (K=65) -> Relu (ACT) -> alpha^T; AV accumulates res^T.
  4. Final projection out^T = Wf res + bias, DMA out^T; host transposes.

All matmuls (S, S^T, AV, projections, tau transpose) are shaped to run in
the PE's full 128x128 tiling mode -- S contracts over K=65 (row 64 of QT
is zero during the S phase, so it adds exact 0), AV uses a 128-column
stationary operand spanning two heads' V (the lower 64 output rows are
ignored), and the tau transpose input is padded to 72 columns.  Mixed
tile modes force PE array drains on every mode switch (~2x matmul cost).

Pipelining (2-deep): head h+2's S matmuls interleave with head h's
S^T/AV matmuls in PE issue order, so head h+1's tau chain (computed from
S(h+1) during slot h-1) is never on the PE critical path.  AV lags S^T by
one k-chunk so Relu latency is hidden.  The tau row DMA is issued from
the scalar engine (the sync queue is busy issuing input-DMA doorbells
early on), and input/output DMAs are chunked/split so transfers spread
across DMA queues and overlap compute.

Matmul dtype: float32r (11-bit mantissa, 4x fp32 rate).  Inputs pre-rounded
to the fp32r grid on the host so S and S^T are bit-consistent.
"""

import numpy as np

N_HEADS = 8
N_DIM = 512
ATTN_DIM = 64
TEMPERATURE = ATTN_DIM ** 0.5
BS = 8
L = 1024

MM_DTYPE_F32R = True

_COMPILED = {}


def _build_nc():
    import concourse.bacc as bacc
    import concourse.mybir as mybir
    import concourse.tile as tile
    from concourse.masks import make_identity

    F32 = mybir.dt.float32
    MMD = mybir.dt.float32r if MM_DTYPE_F32R else F32
    AT = mybir.AluOpType
    AF = mybir.ActivationFunctionType
    AX = mybir.AxisListType

    nc = bacc.Bacc("TRN2", target_bir_lowering=False, debug=False, num_devices=8)

    hqT_d = nc.dram_tensor("hqT", [N_DIM, L], MMD, kind="ExternalInput").ap()
    hkT_d = nc.dram_tensor("hkT", [N_DIM, L], MMD, kind="ExternalInput").ap()
    hvT_d = nc.dram_tensor("hvT", [N_DIM, L], MMD, kind="ExternalInput").ap()
    wqT_d = nc.dram_tensor("wqT", [N_DIM, N_DIM], MMD, kind="ExternalInput").ap()
    wkT_d = nc.dram_tensor("wkT", [N_DIM, N_DIM], MMD, kind="ExternalInput").ap()
    wvT_d = nc.dram_tensor("wvT", [N_DIM, N_DIM], MMD, kind="ExternalInput").ap()
    wfT_d = nc.dram_tensor("wfT", [N_DIM, N_DIM], MMD, kind="ExternalInput").ap()
    bf2_d = nc.dram_tensor("bf2", [N_DIM], F32, kind="ExternalInput").ap()
    rec_d = nc.dram_tensor("recj", [128, 32], F32, kind="ExternalInput").ap()
    outT_d = nc.dram_tensor("outT", [N_DIM, L], F32, kind="ExternalOutput").ap()

    H = N_HEADS
    NQT = L // 128          # 8 q tiles per head
    NKC = L // 128          # 8 k chunks per head
    NDC = N_DIM // 128      # 4 feature chunks
    VW = N_DIM + 64         # v_s free width: 64 zero-pad cols for head 7's AV

    with tile.TileContext(nc) as tc:
        with tc.tile_pool(name="pW", bufs=1) as pW, \
             tc.tile_pool(name="pQK", bufs=1) as pQK, \
             tc.tile_pool(name="pV", bufs=1) as pV, \
             tc.tile_pool(name="pRes", bufs=1) as pRes, \
             tc.tile_pool(name="pOut", bufs=4) as pOut, \
             tc.tile_pool(name="pSm", bufs=1) as pSm, \
             tc.tile_pool(name="pWk", bufs=2) as pWk, \
             tc.tile_pool(name="pA", bufs=3) as pA, \
             tc.tile_pool(name="pIn", bufs=1) as pIn, \
             tc.tile_pool(name="pw3", bufs=1) as pw3, \
             tc.tile_pool(name="psA", bufs=2, space="PSUM") as psA, \
             tc.tile_pool(name="psB", bufs=3, space="PSUM") as psB, \
             tc.tile_pool(name="psR", bufs=1, space="PSUM") as psR, \
             tc.tile_pool(name="psT", bufs=1, space="PSUM") as psT:

            # ---- small constants first ----
            recj = pW.tile([128, 32], F32)
            nc.sync.dma_start(out=recj, in_=rec_d)
            bf2_s = pW.tile([128, NDC], F32)
            nc.sync.dma_start(out=bf2_s, in_=bf2_d.rearrange("(m p) -> p m", p=128))
            identity = pW.tile([128, 128], F32)
            make_identity(nc, identity)

            # ---- input staging (all DMAs issued up front, chunked) ----
            hq_s = pIn.tile([128, NDC, L], MMD)
            hk_s = pIn.tile([128, NDC, L], MMD)
            hv_s = pIn.tile([128, NDC, L], MMD)
            wq_s = pw3.tile([128, NDC, N_DIM], MMD)
            wk_s = pw3.tile([128, NDC, N_DIM], MMD)
            wv_s = pw3.tile([128, NDC, N_DIM], MMD)
            wf_s = pW.tile([128, NDC, N_DIM], MMD)
            hqv = hqT_d.rearrange("(c p) l -> p c l", p=128)
            hkv = hkT_d.rearrange("(c p) l -> p c l", p=128)
            # c=0 feeds the first projection matmuls: halve the big activation
            # chunks so they land on two DMA queues each, longest first
            for half in range(2):
                nc.sync.dma_start(out=hq_s[:, 0, half * 512:(half + 1) * 512],
                                  in_=hqv[:, 0, half * 512:(half + 1) * 512])
                nc.sync.dma_start(out=hk_s[:, 0, half * 512:(half + 1) * 512],
                                  in_=hkv[:, 0, half * 512:(half + 1) * 512])
            nc.sync.dma_start(out=wq_s[:, 0, :],
                              in_=wqT_d.rearrange("(c p) o -> p c o", p=128)[:, 0, :])
            nc.sync.dma_start(out=wk_s[:, 0, :],
                              in_=wkT_d.rearrange("(c p) o -> p c o", p=128)[:, 0, :])
            for c in range(1, NDC):
                nc.sync.dma_start(out=wq_s[:, c, :],
                                  in_=wqT_d.rearrange("(c p) o -> p c o", p=128)[:, c, :])
                nc.sync.dma_start(out=hq_s[:, c, :], in_=hqv[:, c, :])
                nc.sync.dma_start(out=wk_s[:, c, :],
                                  in_=wkT_d.rearrange("(c p) o -> p c o", p=128)[:, c, :])
                nc.sync.dma_start(out=hk_s[:, c, :], in_=hkv[:, c, :])
            for c in range(NDC):
                nc.sync.dma_start(out=wv_s[:, c, :],
                                  in_=wvT_d.rearrange("(c p) o -> p c o", p=128)[:, c, :])
            for n in range(2):
                nc.sync.dma_start(
                    out=hv_s[:, :, n * 512:(n + 1) * 512],
                    in_=hvT_d.rearrange("(c p) l -> p c l", p=128)[:, :, n * 512:(n + 1) * 512])
            for c in range(NDC):
                nc.sync.dma_start(out=wf_s[:, c, :],
                                  in_=wfT_d.rearrange("(c p) o -> p c o", p=128)[:, c, :])

            # ---- persistent tiles ----
            qt65 = [pQK.tile([128, L], MMD, name=f"qt65_{h}") for h in range(H)]
            kt65 = [pQK.tile([128, L], MMD, name=f"kt65_{h}") for h in range(H)]
            for h in range(H):
                nc.gpsimd.memset(kt65[h][64:65, :].bitcast(F32), 1.0)
                nc.gpsimd.memset(qt65[h][64:65, :].bitcast(F32), 0.0)

            v_s = pV.tile([128, NKC, VW], MMD)          # v[k, o] (+64 pad cols)
            nc.gpsimd.memset(v_s[:, :, N_DIM:VW].bitcast(F32), 0.0)
            res_sb = pRes.tile([128, NDC, L], MMD)      # res^T chunked by feature

            # sparsemax working set: double-buffered by head parity.
            # cs*/sf* are 16 wide with a permanently-zero half so cumsum /
            # suffix-sum shift-adds need no boundary copies.
            csA16 = [pSm.tile([128, NQT, 16], F32, name=f"csA16_{p}") for p in range(2)]
            csB16 = [pSm.tile([128, NQT, 16], F32, name=f"csB16_{p}") for p in range(2)]
            sfA16 = [pSm.tile([128, NQT, 16], F32, name=f"sfA16_{p}") for p in range(2)]
            sfB16 = [pSm.tile([128, NQT, 16], F32, name=f"sfB16_{p}") for p in range(2)]
            for p in range(2):
                nc.gpsimd.memset(csA16[p][:, :, 0:8], 0.0)
                nc.gpsimd.memset(csB16[p][:, :, 0:8], 0.0)
                nc.gpsimd.memset(sfA16[p][:, :, 8:16], 0.0)
                nc.gpsimd.memset(sfB16[p][:, :, 8:16], 0.0)
            tj = pSm.tile([128, NQT, 16], F32)
            tauPad = pSm.tile([128, 72], F32)
            nc.gpsimd.memset(tauPad[:, 8:72], 0.0)
            negTauT = pSm.tile([8, 128], MMD)

            # ---- stage 1a: QT / KT projections ----
            copy_flip = [0]

            def psum_copy(dst, src):
                # alternate ACT / DVE for PSUM evacuation
                if copy_flip[0] % 2 == 0:
                    nc.scalar.activation(dst, src, AF.Copy)
                else:
                    nc.vector.tensor_copy(dst, src)
                copy_flip[0] += 1

            def emit_proj(j, n, w_s, h_s, dst):
                pj = psA.tile([128, 512], F32, tag="a", name="projp")
                for c in range(NDC):
                    nc.tensor.matmul(
                        pj,
                        w_s[:, c, j * 128:(j + 1) * 128],
                        h_s[:, c, n * 512:(n + 1) * 512],
                        start=(c == 0), stop=(c == NDC - 1))
                psum_copy(dst[2 * j][0:64, n * 512:(n + 1) * 512], pj[0:64, :])
                psum_copy(dst[2 * j + 1][0:64, n * 512:(n + 1) * 512], pj[64:128, :])

            # per-head S-phase working tiles
            def head_tiles():
                C = pWk.tile([128, NQT, 16], F32, tag="C", name="C")
                negC = pWk.tile([128, NQT, 16], F32, tag="negC", name="negC")
                return C, negC

            def emit_S_qt(h, qt, C, negC):
                """S matmuls (K=65, full PE mode) + maxes for (head h, q-tile qt).
                M1 lands in csA16[h%2][:, qt, 8:16]; M2 for qt-1 is emitted by the
                caller (skewed so the GPSIMD negC has time to land)."""
                par = h % 2
                for kh in range(2):
                    s_ps = psA.tile([128, 512], F32, tag="a", name="s_ps")
                    nc.tensor.matmul(
                        s_ps,
                        qt65[h][0:65, qt * 128:(qt + 1) * 128],
                        kt65[h][0:65, kh * 512:(kh + 1) * 512],
                        start=True, stop=True)
                    nc.vector.max(out=C[:, qt, kh * 8:(kh + 1) * 8], in_=s_ps)
                nc.vector.max(out=csA16[par][:, qt, 8:16], in_=C[:, qt, :])
                nc.gpsimd.tensor_scalar(out=negC[:, qt, :], in0=C[:, qt, :],
                                        scalar1=-1.0, scalar2=None, op0=AT.mult)

            def emit_M2(h, qt, negC):
                nc.vector.max(out=sfA16[h % 2][:, qt, 0:8], in_=negC[:, qt, :])

            def emit_V_kc(kc):
                pv = psA.tile([128, 512], F32, tag="a", name="vp")
                for c in range(NDC):
                    nc.tensor.matmul(
                        pv,
                        hv_s[:, c, kc * 128:(kc + 1) * 128],
                        wv_s[:, c, :],
                        start=(c == 0), stop=(c == NDC - 1))
                nc.scalar.activation(v_s[:, kc, 0:N_DIM], pv, AF.Copy)

            def emit_tau_chain(h):
                """cumsum/suffix (GPSIMD shift-adds), candidates + reduce (DVE)."""
                par = h % 2
                cA, cB, sA, sB = csA16[par], csB16[par], sfA16[par], sfB16[par]
                # cumsum of M1 (data in [:, :, 8:16], zeros to the left)
                nc.gpsimd.tensor_tensor(out=cB[:, :, 8:16], in0=cA[:, :, 8:16],
                                        in1=cA[:, :, 7:15], op=AT.add)
                nc.gpsimd.tensor_tensor(out=cA[:, :, 8:16], in0=cB[:, :, 8:16],
                                        in1=cB[:, :, 6:14], op=AT.add)
                nc.gpsimd.tensor_tensor(out=cB[:, :, 8:16], in0=cA[:, :, 8:16],
                                        in1=cA[:, :, 4:12], op=AT.add)
                # suffix sums of M2 (data in [:, :, 0:8], zeros to the right)
                nc.gpsimd.tensor_tensor(out=sB[:, :, 0:8], in0=sA[:, :, 0:8],
                                        in1=sA[:, :, 1:9], op=AT.add)
                nc.gpsimd.tensor_tensor(out=sA[:, :, 0:8], in0=sB[:, :, 0:8],
                                        in1=sB[:, :, 2:10], op=AT.add)
                nc.gpsimd.tensor_tensor(out=sB[:, :, 0:8], in0=sA[:, :, 0:8],
                                        in1=sA[:, :, 4:12], op=AT.add)
                # tj[0:8]  = (cs_j - 1) * (1/j)
                nc.vector.scalar_tensor_tensor(
                    out=tj[:, :, 0:8], in0=cB[:, :, 8:16], scalar=1.0,
                    in1=recj[:, 0:8].unsqueeze(1).to_broadcast([128, NQT, 8]),
                    op0=AT.subtract, op1=AT.mult)
                # tj[8:16] = (cs_8 - r_p - 1) * 1/(16-p)
                nc.vector.tensor_tensor(
                    out=tj[:, :, 8:16],
                    in0=cB[:, :, 15:16].to_broadcast([128, NQT, 8]),
                    in1=sB[:, :, 0:8], op=AT.subtract)
                nc.vector.scalar_tensor_tensor(
                    out=tj[:, :, 8:16], in0=tj[:, :, 8:16], scalar=1.0,
                    in1=recj[:, 16:24].unsqueeze(1).to_broadcast([128, NQT, 8]),
                    op0=AT.subtract, op1=AT.mult)
                nc.vector.tensor_reduce(out=tauPad[:, 0:8], in_=tj,
                                        axis=AX.X, op=AT.max)

            def emit_tau_plumb(h):
                """PE transpose (128x128 mode via 72-col pad) -> negate -> row 64."""
                tauT_ps = psT.tile([72, 128], F32, tag="tauT", name="tauT")
                nc.tensor.transpose(tauT_ps, tauPad, identity)
                nc.scalar.activation(negTauT, tauT_ps[0:8, :], AF.Copy,
                                     bias=0.0, scale=-1.0)
                nc.scalar.dma_start(out=qt65[h][64:65, :], in_=negTauT)

            def emit_ST_kc(h, kc):
                """S^T - tau (K=65) -> relu -> alpha^T.  Returns alphaT tile.
                Heads 6/7 run in slots with no S-phase interleave (2-deep
                pipeline has nothing left to prefetch), so ACT's two Relus per
                k-chunk would outpace the PE there; route one to the idle DVE."""
                alphaT = pA.tile([128, L], MMD, tag="alphaT", name="alphaT")
                for qh in range(2):
                    st_ps = psB.tile([128, 512], F32, tag="b", name="st_ps")
                    nc.tensor.matmul(
                        st_ps,
                        kt65[h][0:65, kc * 128:(kc + 1) * 128],
                        qt65[h][0:65, qh * 512:(qh + 1) * 512],
                        start=True, stop=True)
                    if h >= H - 2 and qh == 1:
                        nc.vector.tensor_scalar(
                            out=alphaT[:, qh * 512:(qh + 1) * 512], in0=st_ps,
                            scalar1=0.0, scalar2=None, op0=AT.max)
                    else:
                        nc.scalar.activation(alphaT[:, qh * 512:(qh + 1) * 512],
                                             st_ps, AF.Relu)
                return alphaT

            def emit_AV_kc(h, kc, alphaT, res_ps):
                """AV accumulate; 128-col stationary (head h cols 0:64 valid)."""
                for qh in range(2):
                    nc.tensor.matmul(
                        res_ps[:, qh * 512:(qh + 1) * 512],
                        v_s[:, kc, h * 64:h * 64 + 128],
                        alphaT[:, qh * 512:(qh + 1) * 512],
                        start=(kc == 0), stop=(kc == NKC - 1))

            # ---- S-unit emitter with skewed M2 (gives GPSIMD negC time) ----
            pending_m2 = [None]

            def emit_S_unit(h, qt, C, negC):
                if pending_m2[0] is not None:
                    emit_M2(*pending_m2[0])
                emit_S_qt(h, qt, C, negC)
                pending_m2[0] = (h, qt, negC)

            def flush_M2():
                if pending_m2[0] is not None:
                    emit_M2(*pending_m2[0])
                    pending_m2[0] = None

            # ---- prologue: projections with head-0/1 S phases interleaved ----
            # (2-deep pipeline: S(h) runs two slots ahead of B(h) so the tau
            # chain/plumb latency is fully hidden.)
            tiles = {0: head_tiles(), 1: head_tiles()}
            squeue = [(0, q) for q in range(NQT)] + [(1, q) for q in range(NQT)]
            sq = 0
            for n in range(2):
                emit_proj(0, n, wq_s, hq_s, qt65)
                emit_proj(0, n, wk_s, hk_s, kt65)
            for j in range(1, NDC):
                for n in range(2):
                    emit_proj(j, n, wq_s, hq_s, qt65)
                    emit_proj(j, n, wk_s, hk_s, kt65)
                for _ in range(2):
                    hh, qq = squeue[sq]; sq += 1
                    emit_S_unit(hh, qq, *tiles[hh])
            for kc in range(NKC):
                hh, qq = squeue[sq]; sq += 1
                emit_S_unit(hh, qq, *tiles[hh])
                emit_V_kc(kc)
                if kc == 2:
                    emit_tau_chain(0)
                    emit_tau_plumb(0)
            while sq < len(squeue):
                hh, qq = squeue[sq]; sq += 1
                emit_S_unit(hh, qq, *tiles[hh])
            flush_M2()

            # ---- main pipelined loop over heads ----
            for h in range(H):
                res_ps = psR.tile([128, L], F32, tag="res", name="res_ps")
                if h + 2 < H:
                    tiles[h + 2] = head_tiles()
                prev = None  # (kc, alphaT) awaiting AV
                for i in range(NQT):
                    if h + 2 < H:
                        emit_S_unit(h + 2, i, *tiles[h + 2])
                    alphaT = emit_ST_kc(h, i)
                    if prev is not None:
                        emit_AV_kc(h, prev[0], prev[1], res_ps)
                    prev = (i, alphaT)
                    if i == 1 and h + 1 < H:
                        # chain is DVE/GPSIMD-only and safe mid-slot; the plumb
                        # contains a PE transpose which must not land inside the
                        # open AV PSUM accumulation group, so it waits for the
                        # group's stop below
                        emit_tau_chain(h + 1)
                emit_AV_kc(h, prev[0], prev[1], res_ps)
                if h + 1 < H:
                    emit_tau_plumb(h + 1)
                flush_M2()
                half = 64 * (h % 2)
                nc.scalar.activation(res_sb[half:half + 64, h // 2, :],
                                     res_ps[0:64, :], AF.Copy)

            # ---- stage 3: final projection + bias ----
            for m in range(NDC):
                for n in range(2):
                    po = psB.tile([128, 512], F32, tag="b", name="po")
                    for c in range(NDC):
                        nc.tensor.matmul(
                            po,
                            wf_s[:, c, m * 128:(m + 1) * 128],
                            res_sb[:, c, n * 512:(n + 1) * 512],
                            start=(c == 0), stop=(c == NDC - 1))
                    ot = pOut.tile([128, 512], F32, tag="ot", name="ot")
                    nc.vector.tensor_scalar(out=ot, in0=po,
                                            scalar1=bf2_s[:, m:m + 1], scalar2=None,
                                            op0=AT.add)
                    for q in range(2):
                        # alternate doorbell engines: the sync queue serializes
                        # ~0.6us per dma_start; GPSIMD is idle during stage 3
                        eng = nc.sync if q == 0 else nc.gpsimd
                        eng.dma_start(
                            out=outT_d.rearrange("(m p) l -> p m l", p=128)[
                                :, m, n * 512 + q * 256:n * 512 + (q + 1) * 256],
                            in_=ot[:, q * 256:(q + 1) * 256])

    nc.compile()
    return nc


def _round_f32r(x):
    """Round fp32 array to the fp32r grid (11-bit mantissa, round-to-nearest)."""
    if not MM_DTYPE_F32R:
        return np.ascontiguousarray(x, dtype=np.float32)
    v = np.ascontiguousarray(x, dtype=np.float32).view(np.uint32)
    r = ((v.astype(np.uint64) + 0x800) & 0xFFFFF000).astype(np.uint32)
    return r.view(np.float32)


def _prep_inputs(h_q, h_k, h_v, Wq, Wk, Wv, bv, Wf, bf):
    f32 = np.float32
    wqT = _round_f32r((np.asarray(Wq, f32) / TEMPERATURE).T)
    wkT = _round_f32r(np.asarray(Wk, f32).T)
    wvT = _round_f32r(np.asarray(Wv, f32).T)
    wfT = _round_f32r(np.asarray(Wf, f32).T)
    bf2 = (np.asarray(Wf, np.float64) @ np.asarray(bv, np.float64)
           + np.asarray(bf, np.float64)).astype(f32)
    rec = np.zeros(32, dtype=f32)
    rec[0:16] = (1.0 / np.arange(1, 17, dtype=np.float64)).astype(f32)
    rec[16:24] = (1.0 / np.arange(16, 8, -1, dtype=np.float64)).astype(f32)
    recj = np.ascontiguousarray(np.broadcast_to(rec, (128, 32)))
    shared = {"wqT": wqT, "wkT": wkT, "wvT": wvT, "wfT": wfT, "bf2": bf2, "recj": recj}
    in_maps = []
    for b in range(BS):
        m = dict(shared)
        m["hqT"] = _round_f32r(np.asarray(h_q[b], f32).T)
        m["hkT"] = _round_f32r(np.asarray(h_k[b], f32).T)
        m["hvT"] = _round_f32r(np.asarray(h_v[b], f32).T)
        in_maps.append(m)
    return in_maps


def kernel(h_q, h_k, h_v, Wq, Wk, Wv, bv, Wf, bf):
    from concourse.bass_utils import run_bass_kernel_spmd

    if "nc" not in _COMPILED:
        _COMPILED["nc"] = _build_nc()
    nc = _COMPILED["nc"]

    in_maps = _prep_inputs(h_q, h_k, h_v, Wq, Wk, Wv, bv, Wf, bf)
    res = run_bass_kernel_spmd(nc, in_maps, core_ids=list(range(BS)))
    out = np.empty((BS, L, N_DIM), dtype=np.float32)
    for b in range(BS):
        out[b] = res.results[b]["outT"].T
    return out


if __name__ == "__main__":
    rng = np.random.default_rng(0)
    d = N_DIM
    s = 1.0 / np.sqrt(d)
    ins = {
        "h_q": rng.standard_normal((BS, L, d), dtype=np.float32),
        "h_k": rng.standard_normal((BS, L, d), dtype=np.float32),
        "h_v": rng.standard_normal((BS, L, d), dtype=np.float32),
        "Wq": rng.standard_normal((d, d), dtype=np.float32) * s,
        "Wk": rng.standard_normal((d, d), dtype=np.float32) * s,
        "Wv": rng.standard_normal((d, d), dtype=np.float32) * s,
        "bv": rng.standard_normal((d,), dtype=np.float32) * s,
        "Wf": rng.standard_normal((d, d), dtype=np.float32) * s,
        "bf": rng.standard_normal((d,), dtype=np.float32) * s,
    }
    out = kernel(**ins)
    print("kernel ran, out shape", out.shape)
